# revision 11
# baseline (speedup 1.0000x reference)
"""Bass/Tile kernel: cosine top-20 adjacency (16384x64 embeddings) on 8 trn2 cores.

Per-core algorithm (rows sharded 2048/core via host-side input rotation so the
same SPMD graph runs on every core):
  1. Normalize rows (square -> reduce -> sqrt -> recip), fused scale-by-64 +
     fp8e4 cast (STT).  Scaled sims land at 4096*cos in PSUM.
  2. Write fp8 scratch to DRAM, XBAR-transpose the uint16 (fp8-pair) view
     [4096, 128] -> nt8 [128, 4096], then 4 partition-block DMAs assemble
     normT8 [32, 16384] u16: partition p holds dim-pair (2p, 2p+1); u16 col
     q = j*4096 + m corresponds to emb row x = 4*m + j (column permutation is
     irrelevant: output is values-only).
  3. Per 128-row tile: 8 matmuls (2048 cols each) in fp8 DoubleRowSwInterleave
     mode (weights = interleaved pair view, output rows come back reversed;
     host un-permutes).  PSUM drain: 6 groups via Act cast->bf16, 2 groups via
     DVE tensor_max of PSUM halves.  Fold pyramid split DVE/GpSimd down to 512
     window maxima, 4x max8 -> 32 candidates, 3x(max8+match_replace) -> top-24.
  4. Self-similarity (=4096) is always the strict row max, so out[:,0] = 0 and
     out[:,1:20] = sigmoid(top24[:,1:20] / 4096) via the Act scale parameter.
"""

import os
import sys

import numpy as np

for _p in ("/opt/trn_rl_repo",):
    if _p not in sys.path and os.path.isdir(_p):
        sys.path.insert(0, _p)

import concourse.bass as bass  # noqa: E402
import concourse.mybir as mybir  # noqa: E402
import concourse.tile as tile  # noqa: E402
from concourse import bacc  # noqa: E402
from concourse import bass_utils  # noqa: E402
from concourse.bass_utils import run_bass_kernel_spmd  # noqa: E402

# NOTE: walrus --enable-ldw-opt=true (LDWEIGHTS dedup) crashes codegen at
# visitInstLdweights in this build; per-matmul weight loads are unavoidable.

N = 16384
D = 64
TOPK = 20
CORES = 8
R = N // CORES  # 2048 rows per core
T = R // 128  # 16 row tiles per core
G = 2048  # column group size
NG = N // G  # 8 column groups
NEG = -1.0e30
FSCALE = 64.0  # fp8 embedding scale; sims come out x4096

f32 = mybir.dt.float32
bf16 = mybir.dt.bfloat16
fp8 = mybir.dt.float8e4
u16 = mybir.dt.uint16
AF = mybir.ActivationFunctionType
ALU = mybir.AluOpType
PM = mybir.MatmulPerfMode

# issue order of column groups: even groups are ready after transpose chunk 0
GORDER = (0, 2, 4, 6, 1, 3, 5, 7)
# issue slot drained by DVE fused tensor_max (one PSUM operand); rest: Act
V_ISSUE = 2

_CACHE = {}


def _build_nc():
    nc = bacc.Bacc(
        "TRN2", target_bir_lowering=False, debug=False, enable_asserts=False
    )
    emb = nc.dram_tensor("embeddings", [N, D], f32, kind="ExternalInput")
    out = nc.dram_tensor("out", [R, TOPK], f32, kind="ExternalOutput")
    out_v = out[:].rearrange("(t o) k -> t o k", t=T)

    with tile.TileContext(nc) as tc:
        with tc.tile_pool(name="persist", bufs=1) as persist:
            # partition p holds fp8 dim-pair (2p, 2p+1); u16 col q=j*4096+m
            normT8u = persist.tile([32, N], u16)

            # ---- Prologue: normalize+scale rows, fp8 cast, XBAR transpose ----
            with (
                tc.tile_pool(name="pro_rm", bufs=1) as pro_rm,
                tc.tile_pool(name="pro_t2", bufs=1) as pro_t2,
                tc.tile_pool(name="pro_dram", bufs=1, space="DRAM") as pro_dram,
            ):
                emb_v = emb[:].rearrange("(p a) d -> p a d", p=128)
                rm = pro_rm.tile([128, 128, D], f32)
                sq = pro_rm.tile([128, 128, D], f32)
                ssq = pro_rm.tile([128, 128], f32)
                slen = pro_rm.tile([128, 128], f32)
                sinv = pro_rm.tile([128, 128], f32)
                rmb8 = pro_rm.tile([128, 128, D], fp8)
                scratch = pro_dram.tile([N, 32], u16)
                sc_v = scratch[:].bitcast(fp8).rearrange("(p a) d -> p a d", p=128)
                # u16 transpose source: row m holds pairs of emb rows 4m..4m+3
                sc_t = scratch[:].rearrange("(m four) pd -> m (four pd)", four=4)
                nt8 = pro_t2.tile([128, N // 4], u16)
                engs = (nc.sync, nc.scalar)
                NCH = 8
                CW = 128 // NCH
                for c in range(NCH):
                    cs = slice(c * CW, (c + 1) * CW)
                    engs[c % 2].dma_start(rm[:, cs, :], emb_v[:, cs, :])
                    nc.scalar.activation(sq[:, cs, :], rm[:, cs, :], AF.Square)
                    nc.vector.tensor_reduce(
                        ssq[:, cs], sq[:, cs, :],
                        axis=mybir.AxisListType.X, op=ALU.add,
                    )
                    nc.scalar.activation(slen[:, cs], ssq[:, cs], AF.Sqrt)
                    nc.vector.reciprocal(sinv[:, cs], slen[:, cs])
                    nc.vector.scalar_tensor_tensor(
                        rmb8[:, cs, :], rm[:, cs, :], FSCALE,
                        sinv[:, cs].to_broadcast((128, CW, D)),
                        op0=ALU.mult, op1=ALU.mult,
                    )
                    engs[c % 2].dma_start(sc_v[:, cs, :], rmb8[:, cs, :])

                # XBAR transpose (scalar ring only), 2 chunks of [2048, 128];
                # then move partition block j -> partitions 0-31 of normT8u.
                for c in range(2):
                    ms = slice(c * 2048, (c + 1) * 2048)
                    nc.scalar.dma_start(
                        out=nt8[:, ms], in_=sc_t[ms, :], transpose=True
                    )
                    for j in range(4):
                        nc.sync.dma_start(
                            normT8u[0:32, j * 4096 + c * 2048 :
                                    j * 4096 + (c + 1) * 2048],
                            nt8[j * 32 : (j + 1) * 32, ms],
                        )

            # fp8 pair view: [32, q, two] with q u16-strided, two byte-strided
            v3 = normT8u[:].bitcast(fp8).rearrange("p (q two) -> p q two", two=2)

            # ---- Main loop: 16 row tiles ----
            with (
                tc.tile_pool(name="mm_psum", bufs=2, space="PSUM") as mm_psum,
                tc.tile_pool(name="ev_cast", bufs=8) as ev_cast,
                tc.tile_pool(name="pyr", bufs=2) as pyr,
                tc.tile_pool(name="fin", bufs=2) as fin,
            ):
                for t in range(T):
                    # tile rows: emb rows x = 4*(mb*128 + (127-o)) + j
                    j, mb = t // 4, t % 4
                    qt0 = j * 4096 + mb * 128
                    lhsT = v3[:, qt0 : qt0 + 128, :]

                    ca = []
                    vt = None
                    for i, g in enumerate(GORDER):
                        ps = mm_psum.tile([128, G], f32, tag="ps")
                        for s in range(G // 512):
                            q0 = g * G + s * 512
                            rhs = v3[:, q0 : q0 + 512, :].rearrange(
                                "p q two -> p two q"
                            )
                            nc.tensor.matmul(
                                ps[:, s * 512 : (s + 1) * 512], lhsT, rhs,
                                perf_mode=PM.DoubleRowSwInterleave,
                            )
                        if i == V_ISSUE:
                            # fused drain: max(PSUM group, already-drained bf16
                            # group) — one PSUM operand (HW limit), the SBUF
                            # operand's L1 fold rides along for free.
                            vt = pyr.tile([128, G], bf16, tag="vt")
                            nc.vector.tensor_max(vt[:], ps[:], ca[0][:])
                        else:
                            cf = ev_cast.tile([128, G], bf16, tag="ca")
                            nc.scalar.activation(cf[:], ps[:], AF.Copy)
                            ca.append(cf)

                    # fold 7 strips (vt, ca1..ca6) down to 512 windows
                    p1 = pyr.tile([128, G], bf16, tag="p1")
                    nc.vector.tensor_max(p1[:], ca[1][:], ca[2][:])
                    p2 = pyr.tile([128, G], bf16, tag="p2")
                    nc.vector.tensor_max(p2[:], ca[3][:], ca[4][:])
                    p3 = pyr.tile([128, G], bf16, tag="p3")
                    nc.vector.tensor_max(p3[:], ca[5][:], ca[6][:])

                    q1 = pyr.tile([128, G], bf16, tag="q1")
                    nc.vector.tensor_max(q1[:], p1[:], p2[:])
                    q2 = pyr.tile([128, G], bf16, tag="q2")
                    nc.vector.tensor_max(q2[:], p3[:], vt[:])
                    w2 = pyr.tile([128, G], bf16, tag="w2")
                    nc.vector.tensor_max(w2[:], q1[:], q2[:])
                    w1 = pyr.tile([128, G // 2], bf16, tag="w1")
                    nc.vector.tensor_max(w1[:], w2[:, 0:1024], w2[:, 1024:2048])
                    w0 = pyr.tile([128, G // 4], bf16, tag="w0")
                    nc.vector.tensor_max(w0[:], w1[:, 0:512], w1[:, 512:1024])

                    # candidates: top-8 of each 128-chunk of the 512 maxima
                    cand = fin.tile([128, 32], bf16, tag="cand")
                    for c in range(4):
                        nc.vector.max(
                            out=cand[:, c * 8 : (c + 1) * 8],
                            in_=w0[:, c * 128 : (c + 1) * 128],
                        )
                    # top-24 via 3x max8 + 2x match_replace
                    top24 = fin.tile([128, 24], bf16, tag="top24")
                    cand2 = fin.tile([128, 32], bf16, tag="cand2")
                    cand3 = fin.tile([128, 32], bf16, tag="cand3")
                    nc.vector.max(out=top24[:, 0:8], in_=cand[:])
                    nc.vector.match_replace(
                        out=cand2[:], in_to_replace=top24[:, 0:8],
                        in_values=cand[:], imm_value=NEG,
                    )
                    nc.vector.max(out=top24[:, 8:16], in_=cand2[:])
                    nc.vector.match_replace(
                        out=cand3[:], in_to_replace=top24[:, 8:16],
                        in_values=cand2[:], imm_value=NEG,
                    )
                    nc.vector.max(out=top24[:, 16:24], in_=cand3[:])

                    # epilogue: out[:,0]=0; out[:,1:20]=sigmoid(top24[1:20]/4096)
                    osb = fin.tile([128, TOPK], f32, tag="osb")
                    nc.gpsimd.memset(osb[:, 0:1], 0.0)
                    nc.scalar.activation(
                        osb[:, 1:TOPK], top24[:, 1:TOPK], AF.Sigmoid,
                        scale=1.0 / (FSCALE * FSCALE),
                    )
                    nc.sync.dma_start(out_v[t], osb[:])

    nc.compile()
    return nc


def get_nc():
    if "nc" not in _CACHE:
        _CACHE["nc"] = _build_nc()
    return _CACHE["nc"]


def _row_perm():
    """perm[x] = device out index (t*128+o) holding local row x."""
    x = np.arange(R)
    j, m = x % 4, x // 4
    mb, o = m // 128, 127 - (m % 128)
    t = j * 4 + mb
    return t * 128 + o


def kernel(embeddings: np.ndarray) -> np.ndarray:
    emb = np.ascontiguousarray(np.asarray(embeddings, dtype=np.float32))
    assert emb.shape == (N, D), emb.shape
    nc = get_nc()
    in_maps = [
        {"embeddings": np.roll(emb, -i * R, axis=0)} for i in range(CORES)
    ]
    res = run_bass_kernel_spmd(nc, in_maps, core_ids=list(range(CORES)))
    _CACHE["last_results"] = res
    perm = _row_perm()
    return np.concatenate(
        [res.results[i]["out"][perm] for i in range(CORES)], axis=0
    ).astype(np.float32)
